# revision 13
# baseline (speedup 1.0000x reference)
"""Trainium2 Bass kernel for nn_MemorySelfAttention_8890582303066.

Sharding: 8 cores = 2 batches x 4 head-groups (4 heads each, tensor parallel).
w_attn column-sharded, w_proj row-sharded; host reduces the 4 partial outputs
per batch (the unshard step implied by row-sharded w_proj).

Only the last T query rows survive y[:, -T:, :] @ w_proj, so long_q is never
needed and attention runs with just the T x-token queries against all M keys.

On-chip per core:
  B) qkv projection vs the column slice of w_attn; RoPE applied via a
     pair-swap permutation matmul + two table multiplies (tables precomputed
     host-side, input independent).
  C) scores computed TRANSPOSED (keys on partitions, queries free) so softmax
     needs no on-chip transposes: exp without max subtraction (|scaled score|
     <= ~4 for randn inputs), denominator via an appended ones-column in V
     (row 64 of the PV accumulation), normalization folded in at the end.
  D) partial out = Y^T.T @ w_proj_rows, DMA'd out bf16; host sums partials.

v2 schedule: the ACT engine (exp) has ~75us of work and the PE ~86us; the
kernel is limited by how early the exp stream starts and how tightly the PE
stream packs.  Changes vs the first version:
 - DMA priority order delivers the minimal prefix for q-projection + long-key
   scores first (wqk-q, xT x-cols, rope tables x-half, lkT), so the first exp
   fires at ~16us instead of ~22us.
 - attention (score/exp/PV) emitted under tc.high_priority so the scheduler
   treats projection work as filler; k/v jobs emitted just before the
   attention pass that consumes them.
 - per-(qg,hg) kc order visits long keys, then x keys, then stm keys to
   match DMA arrival order.
 - qg1 tail: both head-pairs' normalization uses the fp32 indicator matmul
   (no gpsimd cast on the critical chain), the normalize multiply and output
   projection run per-128-query-block, psum->sbuf drains for the tail run on
   the scalar engine (idle after the last exp), and the output is bf16
   (halves the final DMA).
"""

import numpy as np
import ml_dtypes
BF = ml_dtypes.bfloat16

B, T, C, H, HD, S, L = 2, 1024, 1024, 16, 64, 512, 1024
NX = S + T              # 1536 projected positions (stm + x)
M = L + S + T           # 2560 total keys
THETA = 10000.0
N_CORES = 8

_cache = {}


def _host_tables():
    inv = 1.0 / (THETA ** (np.arange(0, HD, 2, dtype=np.float64) / HD))
    ang = np.outer(np.arange(NX, dtype=np.float64), inv)
    cos_t = np.cos(ang).T.astype(np.float32)          # (32, NX)
    sin_t = np.sin(ang).T.astype(np.float32)
    c64 = np.repeat(cos_t, 2, axis=0)                 # (64, NX)
    s64 = np.repeat(sin_t, 2, axis=0)
    s64[0::2] *= -1.0
    ctab = np.ascontiguousarray(np.tile(c64, (2, 1)))  # (128, NX)
    stab = np.ascontiguousarray(np.tile(s64, (2, 1)))
    pswap = np.zeros((128, 128), np.float32)
    pswap[np.arange(128), np.arange(128) ^ 1] = 1.0
    tri = np.where(np.arange(128)[:, None] <= np.arange(128)[None, :],
                   np.float32(1.0), np.float32(0.0)).astype(np.float32)
    return ctab, stab, pswap, tri


def build_program():
    if "nc" in _cache:
        return _cache["nc"]
    import concourse.bass as bass
    import concourse.tile as tile
    from concourse import bacc, mybir

    F32 = mybir.dt.float32
    BF16 = mybir.dt.bfloat16
    EXP = mybir.ActivationFunctionType.Exp

    nc = bacc.Bacc("TRN2", target_bir_lowering=False, debug=False,
                   num_devices=N_CORES)

    xT_d = nc.dram_tensor("xT", (C, NX), BF16, kind="ExternalInput")
    wqk_d = nc.dram_tensor("wqk", (C, 512), BF16, kind="ExternalInput")
    wv_d = nc.dram_tensor("wv", (C, 256), BF16, kind="ExternalInput")
    wp_d = nc.dram_tensor("wp", (256, C), BF16, kind="ExternalInput")
    lkT_d = nc.dram_tensor("lkT", (2, 128, L), BF16, kind="ExternalInput")
    lv_d = nc.dram_tensor("lv", (8, 128, 4, HD + 1), BF16, kind="ExternalInput")
    ctab_d = nc.dram_tensor("ctab", (128, NX), BF16, kind="ExternalInput")
    stab_d = nc.dram_tensor("stab", (128, NX), BF16, kind="ExternalInput")
    pswap_d = nc.dram_tensor("pswap", (128, 128), BF16, kind="ExternalInput")
    tri_d = nc.dram_tensor("tri", (128, 128), BF16, kind="ExternalInput")
    vones_d = nc.dram_tensor("vones", (128, 48), BF16, kind="ExternalInput")
    ind2_d = nc.dram_tensor("ind2", (33, 128), BF16, kind="ExternalInput")
    ind2f_d = nc.dram_tensor("ind2f", (33, 128), F32, kind="ExternalInput")
    zeros_d = nc.dram_tensor("zeros", (128, 768), BF16, kind="ExternalInput")
    out_d = nc.dram_tensor("out", (T, C), BF16, kind="ExternalOutput")

    with tile.TileContext(nc) as tc, \
         nc.allow_low_precision(reason="bf16 matmul operands"):
        with tc.tile_pool(name="consts", bufs=1) as consts, \
             tc.tile_pool(name="persist", bufs=1) as persist:
            ctab = consts.tile([128, NX], BF16)
            stab = consts.tile([128, NX], BF16)
            pswap = consts.tile([128, 128], BF16)
            tri = consts.tile([128, 128], BF16)
            ind2 = consts.tile([33, 128], BF16)
            ind2f = consts.tile([33, 128], F32)
            # softmax reciprocal staging: rows 0/32 hold 1/denominator for
            # the two heads of a pair, free-dim index = head-pair slot; other
            # rows stay 1.0 so the zero rows of the indicator matmul kill
            # them without NaN risk.
            rdp = consts.tile([33, 2, 512], F32)
            dstage = consts.tile([33, 512], F32)
            zeros = consts.tile([128, 2, 384], BF16)
            vones48 = consts.tile([128, 48], BF16)
            warm = consts.tile([128, 512], F32)
            wp_sb = consts.tile([128, 2, C], BF16)

            kT = persist.tile([128, 2, M], BF16)
            qT = persist.tile([128, 2, T], BF16)
            v_sb = persist.tile([128, 20, 4, HD + 1], BF16)
            yT = persist.tile([128, 2, T], BF16)

            with tc.tile_pool(name="stageB", bufs=1) as sB, \
                 tc.tile_pool(name="rawB", bufs=3) as rawB, \
                 tc.tile_pool(name="ptpool", bufs=8) as ptpool, \
                 tc.tile_pool(name="normC", bufs=2) as normC, \
                 tc.tile_pool(name="obpool", bufs=2) as obpool, \
                 tc.tile_pool(name="psY", bufs=1, space="PSUM") as psY, \
                 tc.tile_pool(name="psS", bufs=2, space="PSUM") as psS, \
                 tc.tile_pool(name="psN", bufs=2, space="PSUM") as psN:
                nc.vector.memset(warm[:], 0.0)
                nc.vector.memset(rdp[:], 1.0)
                nc.vector.memset(dstage[:], 1.0)

                xT = sB.tile([128, 8, NX], BF16)
                wqk = sB.tile([128, 8, 512], BF16)
                wv = sB.tile([128, 8, 256], BF16)
                xT_src = xT_d.ap().rearrange("(a p) n -> p a n", p=128)
                wqk_src = wqk_d.ap().rearrange("(a p) n -> p a n", p=128)
                # DMA priority order = minimal prefix for the exp stream:
                # q weights + x positions + x-half rope tables + long keys
                # first; stm/x2 columns, v weights and late consts after.
                nc.sync.dma_start(wqk[:, :, 0:256], wqk_src[:, :, 0:256])
                nc.sync.dma_start(xT[:, :, 512:768], xT_src[:, :, 512:768])
                nc.sync.dma_start(xT[:, :, 768:1024], xT_src[:, :, 768:1024])
                nc.sync.dma_start(ctab[:, 512:NX], ctab_d.ap()[:, 512:NX])
                nc.sync.dma_start(stab[:, 512:NX], stab_d.ap()[:, 512:NX])
                nc.sync.dma_start(pswap[:], pswap_d.ap())
                nc.sync.dma_start(kT[:, :, 0:L],
                                  lkT_d.ap().rearrange("a p n -> p a n"))
                nc.sync.dma_start(wqk[:, :, 256:512], wqk_src[:, :, 256:512])
                nc.sync.dma_start(v_sb[:, 0:8, :, :],
                                  lv_d.ap().rearrange("c p h d -> p c h d"))
                nc.sync.dma_start(xT[:, :, 0:512], xT_src[:, :, 0:512])
                nc.sync.dma_start(wv[:],
                                  wv_d.ap().rearrange("(a p) n -> p a n", p=128))
                nc.sync.dma_start(ctab[:, 0:512], ctab_d.ap()[:, 0:512])
                nc.sync.dma_start(stab[:, 0:512], stab_d.ap()[:, 0:512])
                nc.sync.dma_start(tri[:], tri_d.ap())
                nc.sync.dma_start(xT[:, :, 1024:1536], xT_src[:, :, 1024:1536])
                nc.sync.dma_start(wp_sb[:],
                                  wp_d.ap().rearrange("(a p) n -> p a n", p=128))
                nc.gpsimd.dma_start(ind2[:], ind2_d.ap())
                nc.gpsimd.dma_start(ind2f[:], ind2f_d.ap())
                nc.gpsimd.dma_start(
                    zeros[:], zeros_d.ap().rearrange("p (a n) -> p a n", a=2))
                nc.gpsimd.dma_start(vones48[:], vones_d.ap())
                nc.vector.tensor_copy(
                    v_sb[:, 8:20, :, HD:HD + 1],
                    vones48[:].rearrange("p (c h d) -> p c h d", c=12, h=4))

                # PE warmup: ramps the HAM clock gate during the DMA-gated
                # window so the q projection starts at 2.4 GHz.
                wps = psN.tile([128, 512], F32, tag="np")
                for wi in range(8):
                    nc.tensor.matmul(wps[:, 0:256], warm[:, 0:128],
                                     warm[:, 0:256],
                                     start=(wi == 0), stop=(wi == 7))

                def emit_proj(kind, pairi, pc, split=False):
                    """q/k projection job for one head-pair and one 512-pos
                    chunk, including RoPE.  With split=True the matmuls AND
                    the rope chain run in two 256-col halves so the first
                    half's rope pipeline starts before the second half's DMA
                    lands (shortens the first-score latency)."""
                    cg = pairi if kind == "q" else (2 + pairi)
                    p1 = psN.tile([128, 512], F32, tag="np")
                    halves = 2 if split else 1
                    hw2 = 512 // halves
                    raw = rawB.tile([128, 512], BF16, tag="raw")
                    p2 = psN.tile([128, 512], F32, tag="np")
                    if kind == "q":
                        dest = qT[:, pairi, (pc - 1) * 512:pc * 512]
                    else:
                        dest = kT[:, pairi, L + pc * 512:L + (pc + 1) * 512]
                    for hv in range(halves):
                        hs = slice(hv * hw2, (hv + 1) * hw2)
                        for c8 in range(8):
                            nc.tensor.matmul(
                                p1[:, hs],
                                wqk[:, c8, cg * 128:(cg + 1) * 128],
                                xT[:, c8, pc * 512 + hv * hw2:
                                   pc * 512 + (hv + 1) * hw2],
                                start=(c8 == 0), stop=(c8 == 7))
                        ts = (slice(0, 128),
                              slice(pc * 512 + hv * hw2,
                                    pc * 512 + (hv + 1) * hw2))
                        nc.vector.tensor_copy(raw[:, hs], p1[:, hs])
                        nc.tensor.matmul(p2[:, hs], pswap[:], raw[:, hs],
                                         start=True, stop=True)
                        nc.vector.tensor_mul(raw[:, hs], raw[:, hs], ctab[ts])
                        nc.vector.tensor_mul(dest[:, hs], p2[:, hs], stab[ts])
                        nc.vector.tensor_add(dest[:, hs], dest[:, hs],
                                             raw[:, hs])

                def emit_v(vpc):
                    pv = psN.tile([128, 4, HD], F32, tag="np")
                    for c8 in range(8):
                        nc.tensor.matmul(
                            pv[:],
                            xT[:, c8, vpc * 128:(vpc + 1) * 128],
                            wv[:, c8, :],
                            start=(c8 == 0), stop=(c8 == 7))
                    nc.vector.tensor_copy(v_sb[:, 8 + vpc, :, 0:HD], pv[:])

                def emit_att(qg, hg, kc_order, deferred):
                    """score -> exp -> PV chain for one (query-half,
                    head-pair); drains psY and computes the softmax
                    reciprocal, deferring the broadcast+multiply."""
                    qs = slice(qg * 512, (qg + 1) * 512)
                    y0 = psY.tile([65, 512], F32, tag="y0")
                    y1 = psY.tile([65, 512], F32, tag="y1")
                    ys = (y0, y1)

                    def emit_score(kc):
                        u = kc - (12 + 4 * qg)
                        c0 = u * 128 if u >= 1 else 0
                        st = psS.tile([128, 2, 512], F32, tag="st")
                        for hh in range(2):
                            po = slice(hh * 64, hh * 64 + 64)
                            nc.tensor.matmul(
                                st[:, hh, c0:512],
                                kT[po, hg, kc * 128:(kc + 1) * 128],
                                qT[po, hg, qg * 512 + c0:(qg + 1) * 512],
                                start=True, stop=True)
                        return st, u

                    # scores emitted one kc ahead of the exp/PV that consumes
                    # them (psS bufs=2 is the matching double buffer).
                    n_kc = len(kc_order)
                    pend = emit_score(kc_order[0])
                    for i, kc in enumerate(kc_order):
                        st, u = pend
                        if i + 1 < n_kc:
                            pend = emit_score(kc_order[i + 1])
                        pt = ptpool.tile([128, 2, 512], BF16, tag="pt")
                        if u >= 1:
                            nc.vector.tensor_copy(pt[:, :, 0:u * 128],
                                                  zeros[:, :, 0:u * 128])
                            nc.scalar.activation(
                                pt[:, :, u * 128:], st[:, :, u * 128:],
                                EXP, scale=0.125)
                        else:
                            nc.scalar.activation(pt[:], st[:], EXP, scale=0.125)
                        if u >= 0:
                            for hh in range(2):
                                blk = slice(u * 128, (u + 1) * 128)
                                nc.vector.tensor_mul(
                                    pt[:, hh, blk], pt[:, hh, blk], tri[:])
                        for hh in range(2):
                            h = hg * 2 + hh
                            nc.tensor.matmul(
                                ys[hh],
                                v_sb[:, kc, h, :],
                                pt[:, hh, :],
                                start=(i == 0), stop=(i == n_kc - 1))
                    # drain psY: denominator rows to the persistent staging
                    # tile and the reciprocal first (they gate the tail's
                    # critical chain), then the big y-row copies.
                    for hh in range(2):
                        nc.vector.tensor_copy(dstage[32 * hh:32 * hh + 1, :],
                                              ys[hh][64:65, :])
                    nc.vector.reciprocal_approx_fast(rdp[:, hg, :],
                                                     dstage[:, :])
                    for hh in range(2):
                        po = slice(hh * 64, hh * 64 + 64)
                        nc.vector.tensor_copy(yT[po, hg, qs], ys[hh][0:64, :])
                    deferred.append(hg)

                HIP = 1 << 20
                LOW = -1500

                # Emission order = scheduler priority for the greedy
                # per-engine dispatch, so projection jobs are ordered by the
                # deadline of their consumers in the exp chain; v jobs (soft
                # deadlines thanks to the 8-deep pt pool) are demoted below
                # the k/q jobs emitted after them.
                emit_proj("q", 0, 1, split=True)
                emit_proj("q", 1, 1, split=True)
                emit_proj("k", 0, 1)
                emit_proj("k", 0, 0)
                with tc.high_priority(offset=LOW):
                    for vpc in (4, 5, 6, 7, 0, 1, 2, 3):
                        emit_v(vpc)

                # kc visit order matches DMA arrival: long keys, x keys
                # (pc1), stm keys (pc0).
                qg0_order = list(range(8)) + [12, 13, 14, 15] + [8, 9, 10, 11]

                deferred0 = []
                with tc.high_priority(offset=HIP):
                    emit_att(0, 0, qg0_order, deferred0)
                emit_proj("k", 1, 1)
                emit_proj("k", 1, 0)
                emit_proj("q", 0, 2)
                with tc.high_priority(offset=HIP):
                    emit_att(0, 1, qg0_order, deferred0)
                emit_proj("q", 1, 2)
                emit_proj("k", 0, 2)
                emit_proj("k", 1, 2)

                # qg0 normalization + output projection (filler priority):
                # bf16 indicator broadcast via a gpsimd cast, as before.
                for hg in deferred0:
                    rdb = normC.tile([33, 512], BF16, tag="rdb")
                    nc.gpsimd.tensor_copy(rdb[:], rdp[:, hg, :])
                    rb = psN.tile([128, 512], F32, tag="np")
                    nc.tensor.matmul(rb[:], ind2[:], rdb[:],
                                     start=True, stop=True)
                    nc.vector.tensor_mul(yT[:, hg, 0:512], yT[:, hg, 0:512],
                                         rb[:])
                for qc in range(4):
                    ob = obpool.tile([128, C], BF16, tag="ob")
                    for ncol in range(2):
                        pd = psN.tile([128, 512], F32, tag="np")
                        for hc in range(2):
                            nc.tensor.matmul(
                                pd[:],
                                yT[:, hc, qc * 128:(qc + 1) * 128],
                                wp_sb[:, hc, ncol * 512:(ncol + 1) * 512],
                                start=(hc == 0), stop=(hc == 1))
                        nc.vector.tensor_copy(
                            ob[:, ncol * 512:(ncol + 1) * 512], pd[:])
                        nc.sync.dma_start(
                            out_d.ap()[qc * 128:(qc + 1) * 128,
                                       ncol * 512:(ncol + 1) * 512],
                            ob[:, ncol * 512:(ncol + 1) * 512])

                # ---- qg1 ----
                qg1_order = list(range(20))
                qs1 = slice(512, 1024)
                deferred1 = []
                with tc.high_priority(offset=LOW):
                    for vpc in (8, 9, 10, 11):
                        emit_v(vpc)
                with tc.high_priority(offset=HIP):
                    emit_att(1, 0, qg1_order, deferred1)
                    # hg0's normalization runs during hg1's attention: fp32
                    # indicator broadcast (borrowing the y0 psum bank) + full
                    # multiply, keeping only hg1's chain on the tail.
                    rb0 = psY.tile([128, 512], F32, tag="y0")
                    nc.tensor.matmul(rb0[:], ind2f[:], rdp[:, 0, :],
                                     start=True, stop=True)
                    nc.vector.tensor_mul(yT[:, 0, qs1], yT[:, 0, qs1],
                                         rb0[:])
                with tc.high_priority(offset=HIP):
                    emit_att(1, 1, qg1_order, deferred1)

                # PE warm-keepers: anchored on the last psY drain so they
                # fill the norm-chain window and stop the clock gate from
                # re-throttling before the output projection.
                warm_ps = psS.tile([128, 512], F32, tag="st")
                for wi in range(8):
                    nc.tensor.matmul(warm_ps[:], zeros[:, 0, 0:128],
                                     yT[:, 1, qs1],
                                     start=(wi == 0), stop=(wi == 7))

                # qg1 tail (critical path): hg1 broadcast, per-128-query
                # normalize + output projection with 4 psum slots and
                # alternating scalar/vector psum drains, bf16 output.
                with tc.high_priority(offset=HIP):
                    rb1 = psY.tile([128, 512], F32, tag="y1")
                    nc.tensor.matmul(rb1[:], ind2f[:], rdp[:, 1, :],
                                     start=True, stop=True)
                    unit = 0
                    for qc in range(4, 8):
                        qb = slice(qc * 128, (qc + 1) * 128)
                        rbb = slice((qc - 4) * 128, (qc - 3) * 128)
                        nc.vector.tensor_mul(yT[:, 1, qb], yT[:, 1, qb],
                                             rb1[:, rbb])
                        ob = obpool.tile([128, C], BF16, tag="ob")
                        for ncol in range(2):
                            pool = psN if unit % 2 == 0 else psS
                            tag = "np" if unit % 2 == 0 else "st"
                            pd = pool.tile([128, 512], F32, tag=tag)
                            for hc in range(2):
                                nc.tensor.matmul(
                                    pd[:],
                                    yT[:, hc, qb],
                                    wp_sb[:, hc, ncol * 512:(ncol + 1) * 512],
                                    start=(hc == 0), stop=(hc == 1))
                            obs = ob[:, ncol * 512:(ncol + 1) * 512]
                            if unit % 2 == 0:
                                nc.scalar.copy(obs, pd[:])
                            else:
                                nc.vector.tensor_copy(obs, pd[:])
                            nc.sync.dma_start(
                                out_d.ap()[qb,
                                           ncol * 512:(ncol + 1) * 512],
                                obs)
                            unit += 1

    nc.compile()
    _cache["nc"] = nc
    return nc


def prep_in_maps(x, short_term_memory, long_k, long_v, w_attn, w_proj):
    ctab, stab, pswap, tri = _host_tables()
    wa = np.ascontiguousarray(w_attn).reshape(C, 3, H, HD)
    in_maps = []
    for core in range(N_CORES):
        b, g = core // 4, core % 4
        hs = slice(4 * g, 4 * g + 4)
        xcat = np.concatenate([short_term_memory[b], x[b]], 0)
        xT = np.ascontiguousarray(xcat.T).astype(BF)
        wk = wa[:, 1, hs, :].reshape(C, 256)
        wq = wa[:, 0, hs, :].reshape(C, 256)
        wqk = np.ascontiguousarray(np.concatenate([wq, wk], 1)).astype(BF)
        wv = np.ascontiguousarray(wa[:, 2, hs, :].reshape(C, 256)).astype(BF)
        lkT = np.ascontiguousarray(
            long_k[b][:, hs, :].transpose(1, 2, 0).reshape(2, 128, L)).astype(BF)
        lv_aug = np.ones((8, 128, 4, HD + 1), BF)
        lv_aug[..., :HD] = long_v[b][:, hs, :].reshape(8, 128, 4, HD).astype(BF)
        wp = np.ascontiguousarray(w_proj[4 * g * 64:(4 * g + 4) * 64, :]).astype(BF)
        ind2 = np.zeros((33, 128), BF)
        ind2[0, 0:64] = 1.0
        ind2[32, 64:128] = 1.0
        in_maps.append({
            "xT": xT, "wqk": wqk, "wv": wv, "wp": wp, "lkT": lkT,
            "lv": lv_aug, "ctab": ctab.astype(BF), "stab": stab.astype(BF),
            "pswap": pswap.astype(BF),
            "tri": tri.astype(BF), "vones": np.ones((128, 48), BF),
            "ind2": ind2,
            "ind2f": ind2.astype(np.float32),
            "zeros": np.zeros((128, 768), BF),
        })
    return in_maps


def kernel(x, short_term_memory, long_q, long_k, long_v, w_attn, w_proj):
    x = np.asarray(x, np.float32)
    short_term_memory = np.asarray(short_term_memory, np.float32)
    long_k = np.asarray(long_k, np.float32)
    long_v = np.asarray(long_v, np.float32)
    w_attn = np.asarray(w_attn, np.float32)
    w_proj = np.asarray(w_proj, np.float32)

    nc = build_program()
    in_maps = prep_in_maps(x, short_term_memory, long_k, long_v, w_attn, w_proj)

    from concourse import bass_utils
    res = bass_utils.run_bass_kernel_spmd(nc, in_maps, core_ids=list(range(N_CORES)))

    out = np.zeros((B, T, C), np.float32)
    for core in range(N_CORES):
        out[core // 4] += res.results[core]["out"].astype(np.float32)
    return out


# revision 19
# speedup vs baseline: 1.1404x; 1.1404x over previous
"""Trainium2 Bass kernel for nn_MemorySelfAttention_8890582303066.

Sharding: 8 cores = 2 batches x 4 head-groups (4 heads each, tensor parallel).
w_attn column-sharded, w_proj row-sharded; host reduces the 4 partial outputs
per batch (the unshard step implied by row-sharded w_proj).

Only the last T query rows survive y[:, -T:, :] @ w_proj, so long_q is never
needed and attention runs with just the T x-token queries against all M keys.

On-chip per core:
  B) qkv projection vs the column slice of w_attn; RoPE applied via a
     pair-swap permutation matmul + two table multiplies (tables precomputed
     host-side, input independent).
  C) scores computed TRANSPOSED (keys on partitions, queries free) so softmax
     needs no on-chip transposes: exp without max subtraction (|scaled score|
     <= ~4 for randn inputs), denominator via an appended ones-column in V
     (row 64 of the PV accumulation), normalization folded in at the end.
  D) partial out = Y^T.T @ w_proj_rows, DMA'd out bf16; host sums partials.

v2 schedule: the ACT engine (exp) has ~75us of work and the PE ~86us; the
kernel is limited by how early the exp stream starts and how tightly the PE
stream packs.  Changes vs the first version:
 - DMA priority order delivers the minimal prefix for q-projection + long-key
   scores first (wqk-q, xT x-cols, rope tables x-half, lkT), so the first exp
   fires at ~16us instead of ~22us.
 - attention (score/exp/PV) emitted under tc.high_priority so the scheduler
   treats projection work as filler; k/v jobs emitted just before the
   attention pass that consumes them.
 - per-(qg,hg) kc order visits long keys, then x keys, then stm keys to
   match DMA arrival order.
 - qg1 tail: both head-pairs' normalization uses the fp32 indicator matmul
   (no gpsimd cast on the critical chain), the normalize multiply and output
   projection run per-128-query-block, psum->sbuf drains for the tail run on
   the scalar engine (idle after the last exp), and the output is bf16
   (halves the final DMA).
"""

import numpy as np
import ml_dtypes
BF = ml_dtypes.bfloat16

B, T, C, H, HD, S, L = 2, 1024, 1024, 16, 64, 512, 1024
NX = S + T              # 1536 projected positions (stm + x)
M = L + S + T           # 2560 total keys
THETA = 10000.0
N_CORES = 8

_cache = {}


def _host_tables():
    inv = 1.0 / (THETA ** (np.arange(0, HD, 2, dtype=np.float64) / HD))
    ang = np.outer(np.arange(NX, dtype=np.float64), inv)
    cos_t = np.cos(ang).T.astype(np.float32)          # (32, NX)
    sin_t = np.sin(ang).T.astype(np.float32)
    c64 = np.repeat(cos_t, 2, axis=0)                 # (64, NX)
    s64 = np.repeat(sin_t, 2, axis=0)
    s64[0::2] *= -1.0
    ctab = np.ascontiguousarray(np.tile(c64, (2, 1)))  # (128, NX)
    stab = np.ascontiguousarray(np.tile(s64, (2, 1)))
    pswap = np.zeros((128, 128), np.float32)
    pswap[np.arange(128), np.arange(128) ^ 1] = 1.0
    tri = np.where(np.arange(128)[:, None] <= np.arange(128)[None, :],
                   np.float32(1.0), np.float32(0.0)).astype(np.float32)
    return ctab, stab, pswap, tri


def build_program():
    if "nc" in _cache:
        return _cache["nc"]
    import concourse.bass as bass
    import concourse.tile as tile
    from concourse import bacc, mybir

    F32 = mybir.dt.float32
    BF16 = mybir.dt.bfloat16
    EXP = mybir.ActivationFunctionType.Exp

    nc = bacc.Bacc("TRN2", target_bir_lowering=False, debug=False,
                   num_devices=N_CORES)

    xT_d = nc.dram_tensor("xT", (C, NX), BF16, kind="ExternalInput")
    wqk_d = nc.dram_tensor("wqk", (C, 512), BF16, kind="ExternalInput")
    wv_d = nc.dram_tensor("wv", (C, 256), BF16, kind="ExternalInput")
    wp_d = nc.dram_tensor("wp", (256, C), BF16, kind="ExternalInput")
    lkT_d = nc.dram_tensor("lkT", (2, 128, L), BF16, kind="ExternalInput")
    lv_d = nc.dram_tensor("lv", (8, 128, 4, HD + 1), BF16, kind="ExternalInput")
    ctab_d = nc.dram_tensor("ctab", (128, NX), BF16, kind="ExternalInput")
    stab_d = nc.dram_tensor("stab", (128, NX), BF16, kind="ExternalInput")
    pswap_d = nc.dram_tensor("pswap", (128, 128), BF16, kind="ExternalInput")
    tri_d = nc.dram_tensor("tri", (128, 128), BF16, kind="ExternalInput")
    vones_d = nc.dram_tensor("vones", (128, 48), BF16, kind="ExternalInput")
    ind2_d = nc.dram_tensor("ind2", (33, 128), BF16, kind="ExternalInput")
    ind2f_d = nc.dram_tensor("ind2f", (33, 128), F32, kind="ExternalInput")
    zeros_d = nc.dram_tensor("zeros", (128, 768), BF16, kind="ExternalInput")
    out_d = nc.dram_tensor("out", (T, C), BF16, kind="ExternalOutput")

    with tile.TileContext(nc) as tc, \
         nc.allow_low_precision(reason="bf16 matmul operands"):
        with tc.tile_pool(name="consts", bufs=1) as consts, \
             tc.tile_pool(name="persist", bufs=1) as persist:
            ctab = consts.tile([128, NX], BF16)
            stab = consts.tile([128, NX], BF16)
            pswap = consts.tile([128, 128], BF16)
            tri = consts.tile([128, 128], BF16)
            ind2 = consts.tile([33, 128], BF16)
            ind2f = consts.tile([33, 128], F32)
            # softmax reciprocal staging: rows 0/32 hold 1/denominator for
            # the two heads of a pair, free-dim index = head-pair slot; other
            # rows stay 1.0 so the zero rows of the indicator matmul kill
            # them without NaN risk.
            rdp = consts.tile([33, 2, 512], F32)
            dstage = consts.tile([33, 2, 512], F32)
            zeros = consts.tile([128, 2, 384], BF16)
            vones48 = consts.tile([128, 48], BF16)
            warm = consts.tile([128, 512], F32)
            wp_sb = consts.tile([128, 2, C], BF16)

            kT = persist.tile([128, 2, M], BF16)
            qT = persist.tile([128, 2, T], BF16)
            v_sb = persist.tile([128, 20, 4, HD + 1], BF16)
            yT = persist.tile([128, 2, T], BF16)

            with tc.tile_pool(name="stageB", bufs=1) as sB, \
                 tc.tile_pool(name="rawB", bufs=3) as rawB, \
                 tc.tile_pool(name="ptpool", bufs=8) as ptpool, \
                 tc.tile_pool(name="normC", bufs=2) as normC, \
                 tc.tile_pool(name="obpool", bufs=2) as obpool, \
                 tc.tile_pool(name="psY", bufs=1, space="PSUM") as psY, \
                 tc.tile_pool(name="psS", bufs=2, space="PSUM") as psS, \
                 tc.tile_pool(name="psN", bufs=2, space="PSUM") as psN:
                nc.vector.memset(warm[:], 0.0)
                nc.vector.memset(rdp[:], 1.0)
                nc.vector.memset(dstage[:], 1.0)

                xT = sB.tile([128, 8, NX], BF16)
                wqk = sB.tile([128, 8, 512], BF16)
                wv = sB.tile([128, 8, 256], BF16)
                xT_src = xT_d.ap().rearrange("(a p) n -> p a n", p=128)
                wqk_src = wqk_d.ap().rearrange("(a p) n -> p a n", p=128)
                # DMA priority order = minimal prefix for the exp stream:
                # q weights + x positions + x-half rope tables + long keys
                # first; stm/x2 columns, v weights and late consts after.
                nc.sync.dma_start(wqk[:, :, 0:256], wqk_src[:, :, 0:256])
                nc.sync.dma_start(xT[:, :, 512:768], xT_src[:, :, 512:768])
                nc.sync.dma_start(xT[:, :, 768:1024], xT_src[:, :, 768:1024])
                nc.sync.dma_start(ctab[:, 512:NX], ctab_d.ap()[:, 512:NX])
                nc.sync.dma_start(stab[:, 512:NX], stab_d.ap()[:, 512:NX])
                nc.sync.dma_start(pswap[:], pswap_d.ap())
                nc.sync.dma_start(kT[:, :, 0:L],
                                  lkT_d.ap().rearrange("a p n -> p a n"))
                nc.sync.dma_start(wqk[:, :, 256:512], wqk_src[:, :, 256:512])
                nc.sync.dma_start(v_sb[:, 0:8, :, :],
                                  lv_d.ap().rearrange("c p h d -> p c h d"))
                nc.sync.dma_start(xT[:, :, 0:512], xT_src[:, :, 0:512])
                nc.sync.dma_start(wv[:],
                                  wv_d.ap().rearrange("(a p) n -> p a n", p=128))
                nc.sync.dma_start(ctab[:, 0:512], ctab_d.ap()[:, 0:512])
                nc.sync.dma_start(stab[:, 0:512], stab_d.ap()[:, 0:512])
                nc.sync.dma_start(tri[:], tri_d.ap())
                nc.sync.dma_start(xT[:, :, 1024:1536], xT_src[:, :, 1024:1536])
                nc.sync.dma_start(wp_sb[:],
                                  wp_d.ap().rearrange("(a p) n -> p a n", p=128))
                nc.gpsimd.dma_start(ind2[:], ind2_d.ap())
                nc.gpsimd.dma_start(ind2f[:], ind2f_d.ap())
                nc.gpsimd.dma_start(
                    zeros[:], zeros_d.ap().rearrange("p (a n) -> p a n", a=2))
                nc.gpsimd.dma_start(vones48[:], vones_d.ap())
                nc.vector.tensor_copy(
                    v_sb[:, 8:20, :, HD:HD + 1],
                    vones48[:].rearrange("p (c h d) -> p c h d", c=12, h=4))

                # PE warmup: ramps the HAM clock gate during the DMA-gated
                # window so the q projection starts at 2.4 GHz.
                wps = psN.tile([128, 512], F32, tag="np")
                for wi in range(8):
                    nc.tensor.matmul(wps[:, 0:256], warm[:, 0:128],
                                     warm[:, 0:256],
                                     start=(wi == 0), stop=(wi == 7))

                def emit_proj(kind, pairi, pc, split=False):
                    """q/k projection job for one head-pair and one 512-pos
                    chunk, including RoPE.  With split=True the matmuls AND
                    the rope chain run in two 256-col halves so the first
                    half's rope pipeline starts before the second half's DMA
                    lands (shortens the first-score latency)."""
                    cg = pairi if kind == "q" else (2 + pairi)
                    p1 = psN.tile([128, 512], F32, tag="np")
                    halves = 2 if split else 1
                    hw2 = 512 // halves
                    raw = rawB.tile([128, 512], BF16, tag="raw")
                    p2 = psN.tile([128, 512], F32, tag="np")
                    if kind == "q":
                        dest = qT[:, pairi, (pc - 1) * 512:pc * 512]
                    else:
                        dest = kT[:, pairi, L + pc * 512:L + (pc + 1) * 512]
                    for hv in range(halves):
                        hs = slice(hv * hw2, (hv + 1) * hw2)
                        for c8 in range(8):
                            nc.tensor.matmul(
                                p1[:, hs],
                                wqk[:, c8, cg * 128:(cg + 1) * 128],
                                xT[:, c8, pc * 512 + hv * hw2:
                                   pc * 512 + (hv + 1) * hw2],
                                start=(c8 == 0), stop=(c8 == 7))
                        ts = (slice(0, 128),
                              slice(pc * 512 + hv * hw2,
                                    pc * 512 + (hv + 1) * hw2))
                        nc.vector.tensor_copy(raw[:, hs], p1[:, hs])
                        nc.tensor.matmul(p2[:, hs], pswap[:], raw[:, hs],
                                         start=True, stop=True)
                        nc.vector.tensor_mul(raw[:, hs], raw[:, hs], ctab[ts])
                        nc.vector.tensor_mul(dest[:, hs], p2[:, hs], stab[ts])
                        nc.vector.tensor_add(dest[:, hs], dest[:, hs],
                                             raw[:, hs])

                def emit_v(vpc):
                    pv = psN.tile([128, 4, HD], F32, tag="np")
                    for c8 in range(8):
                        nc.tensor.matmul(
                            pv[:],
                            xT[:, c8, vpc * 128:(vpc + 1) * 128],
                            wv[:, c8, :],
                            start=(c8 == 0), stop=(c8 == 7))
                    nc.vector.tensor_copy(v_sb[:, 8 + vpc, :, 0:HD], pv[:])

                def emit_att(qg, hg, kc_order, deferred, last=False):
                    """score -> exp -> PV chain for one (query-half,
                    head-pair); drains psY and computes the softmax
                    reciprocal, deferring the broadcast+multiply."""
                    qs = slice(qg * 512, (qg + 1) * 512)
                    y0 = psY.tile([65, 512], F32, tag="y0")
                    y1 = psY.tile([65, 512], F32, tag="y1")
                    ys = (y0, y1)

                    def emit_score(kc):
                        u = kc - (12 + 4 * qg)
                        c0 = u * 128 if u >= 1 else 0
                        st = psS.tile([128, 2, 512], F32, tag="st")
                        for hh in range(2):
                            po = slice(hh * 64, hh * 64 + 64)
                            nc.tensor.matmul(
                                st[:, hh, c0:512],
                                kT[po, hg, kc * 128:(kc + 1) * 128],
                                qT[po, hg, qg * 512 + c0:(qg + 1) * 512],
                                start=True, stop=True)
                        return st, u

                    # scores emitted one kc ahead of the exp/PV that consumes
                    # them (psS bufs=2 is the matching double buffer).
                    n_kc = len(kc_order)
                    pend = emit_score(kc_order[0])
                    for i, kc in enumerate(kc_order):
                        st, u = pend
                        if i + 1 < n_kc:
                            pend = emit_score(kc_order[i + 1])
                        pt = ptpool.tile([128, 2, 512], BF16, tag="pt")
                        c0 = u * 128 if u >= 1 else 0
                        nc.scalar.activation(pt[:, :, c0:], st[:, :, c0:],
                                             EXP, scale=0.125)
                        if u >= 0:
                            for hh in range(2):
                                blk = slice(u * 128, (u + 1) * 128)
                                nc.vector.tensor_mul(
                                    pt[:, hh, blk], pt[:, hh, blk], tri[:])
                        # the fully-masked query prefix is skipped in the PV
                        # matmul too (those psY columns simply take no
                        # contribution from this key block).
                        for hh in range(2):
                            h = hg * 2 + hh
                            nc.tensor.matmul(
                                ys[hh][:, c0:],
                                v_sb[:, kc, h, :],
                                pt[:, hh, c0:],
                                start=(i == 0), stop=(i == n_kc - 1))
                    # drain psY: denominator rows to the persistent staging
                    # tile and the reciprocal first (they gate the tail's
                    # critical chain), then the big y-row copies.  On the
                    # last pass the two copies run on different engines (the
                    # scalar engine is idle once the exps are done).
                    if last:
                        nc.scalar.copy(dstage[0:1, hg, :], ys[0][64:65, :])
                    else:
                        nc.vector.tensor_copy(dstage[0:1, hg, :],
                                              ys[0][64:65, :])
                    nc.vector.tensor_copy(dstage[32:33, hg, :],
                                          ys[1][64:65, :])
                    nc.vector.reciprocal_approx_fast(rdp[:, hg, :],
                                                     dstage[:, hg, :])
                    for hh in range(2):
                        po = slice(hh * 64, hh * 64 + 64)
                        nc.vector.tensor_copy(yT[po, hg, qs], ys[hh][0:64, :])
                    deferred.append(hg)

                HIP = 1 << 20

                # All projection jobs emitted up-front in consumer-deadline
                # order (= filler priority for the greedy per-engine
                # dispatch); the attention streams always outrank them via
                # the high-priority offset.
                emit_proj("q", 0, 1, split=True)
                emit_proj("q", 1, 1, split=True)
                emit_proj("k", 0, 1)
                emit_proj("k", 0, 0)
                for vpc in (4, 5, 6, 7, 0, 1, 2, 3):
                    emit_v(vpc)
                emit_proj("k", 1, 1)
                emit_proj("k", 1, 0)
                emit_proj("q", 0, 2)
                emit_proj("q", 1, 2)
                emit_proj("k", 0, 2)
                for vpc in (8, 9, 10, 11):
                    emit_v(vpc)
                emit_proj("k", 1, 2)

                # kc visit order matches DMA arrival: long keys, x keys
                # (pc1), stm keys (pc0).
                qg0_order = list(range(8)) + [12, 13, 14, 15] + [8, 9, 10, 11]

                deferred0 = []
                with tc.high_priority(offset=HIP):
                    emit_att(0, 0, qg0_order, deferred0)
                with tc.high_priority(offset=HIP):
                    emit_att(0, 1, qg0_order, deferred0)

                # qg0 normalization + output projection (filler priority):
                # bf16 indicator broadcast via a gpsimd cast, as before.
                for hg in deferred0:
                    rdb = normC.tile([33, 512], BF16, tag="rdb")
                    nc.gpsimd.tensor_copy(rdb[:], rdp[:, hg, :])
                    rb = psN.tile([128, 512], F32, tag="np")
                    nc.tensor.matmul(rb[:], ind2[:], rdb[:],
                                     start=True, stop=True)
                    nc.vector.tensor_mul(yT[:, hg, 0:512], yT[:, hg, 0:512],
                                         rb[:])
                for qc in range(4):
                    ob = obpool.tile([128, C], BF16, tag="ob")
                    for ncol in range(2):
                        pd = psN.tile([128, 512], F32, tag="np")
                        for hc in range(2):
                            nc.tensor.matmul(
                                pd[:],
                                yT[:, hc, qc * 128:(qc + 1) * 128],
                                wp_sb[:, hc, ncol * 512:(ncol + 1) * 512],
                                start=(hc == 0), stop=(hc == 1))
                        nc.vector.tensor_copy(
                            ob[:, ncol * 512:(ncol + 1) * 512], pd[:])
                        nc.sync.dma_start(
                            out_d.ap()[qc * 128:(qc + 1) * 128,
                                       ncol * 512:(ncol + 1) * 512],
                            ob[:, ncol * 512:(ncol + 1) * 512])

                # ---- qg1 ----
                qg1_order = list(range(16)) + [17, 18, 19, 16]
                qs1 = slice(512, 1024)
                deferred1 = []
                with tc.high_priority(offset=HIP):
                    emit_att(1, 0, qg1_order, deferred1)
                with tc.high_priority(offset=HIP):
                    emit_att(1, 1, qg1_order, deferred1, last=True)

                # PE warm-keepers: anchored on the last psY drain so they
                # fill the norm-chain window and stop the clock gate from
                # re-throttling before the output projection.
                warm_ps = psS.tile([128, 512], F32, tag="st")
                for wi in range(8):
                    nc.tensor.matmul(warm_ps[:], zeros[:, 0, 0:128],
                                     yT[:, 1, qs1],
                                     start=(wi == 0), stop=(wi == 7))

                # qg1 tail (critical path): bf16 indicator broadcast for
                # both pairs off one cast, per-128-query normalize + output
                # projection with 4 psum slots and alternating scalar/vector
                # psum drains, bf16 output.
                with tc.high_priority(offset=HIP):
                    rdpb = normC.tile([33, 2, 512], BF16, tag="rdpb")
                    nc.vector.tensor_copy(rdpb[:], rdp[:])
                    rbs = []
                    for hg in range(2):
                        rb = psY.tile([128, 512], F32, tag=("y0", "y1")[hg])
                        nc.tensor.matmul(rb[:], ind2[:], rdpb[:, hg, :],
                                         start=True, stop=True)
                        rbs.append(rb)
                    nc.vector.tensor_mul(yT[:, 0, qs1], yT[:, 0, qs1],
                                         rbs[0][:])
                    unit = 0
                    for qc in range(4, 8):
                        qb = slice(qc * 128, (qc + 1) * 128)
                        rbb = slice((qc - 4) * 128, (qc - 3) * 128)
                        nc.vector.tensor_mul(yT[:, 1, qb], yT[:, 1, qb],
                                             rbs[1][:, rbb])
                        ob = obpool.tile([128, C], BF16, tag="ob")
                        for ncol in range(2):
                            pool = psN if unit % 2 == 0 else psS
                            tag = "np" if unit % 2 == 0 else "st"
                            pd = pool.tile([128, 512], F32, tag=tag)
                            for hc in range(2):
                                nc.tensor.matmul(
                                    pd[:],
                                    yT[:, hc, qb],
                                    wp_sb[:, hc, ncol * 512:(ncol + 1) * 512],
                                    start=(hc == 0), stop=(hc == 1))
                            obs = ob[:, ncol * 512:(ncol + 1) * 512]
                            if unit % 2 == 0:
                                nc.scalar.copy(obs, pd[:])
                            else:
                                nc.vector.tensor_copy(obs, pd[:])
                            nc.sync.dma_start(
                                out_d.ap()[qb,
                                           ncol * 512:(ncol + 1) * 512],
                                obs)
                            unit += 1

    nc.compile()
    _cache["nc"] = nc
    return nc


def prep_in_maps(x, short_term_memory, long_k, long_v, w_attn, w_proj):
    ctab, stab, pswap, tri = _host_tables()
    wa = np.ascontiguousarray(w_attn).reshape(C, 3, H, HD)
    in_maps = []
    for core in range(N_CORES):
        b, g = core // 4, core % 4
        hs = slice(4 * g, 4 * g + 4)
        xcat = np.concatenate([short_term_memory[b], x[b]], 0)
        xT = np.ascontiguousarray(xcat.T).astype(BF)
        wk = wa[:, 1, hs, :].reshape(C, 256)
        wq = wa[:, 0, hs, :].reshape(C, 256)
        wqk = np.ascontiguousarray(np.concatenate([wq, wk], 1)).astype(BF)
        wv = np.ascontiguousarray(wa[:, 2, hs, :].reshape(C, 256)).astype(BF)
        lkT = np.ascontiguousarray(
            long_k[b][:, hs, :].transpose(1, 2, 0).reshape(2, 128, L)).astype(BF)
        lv_aug = np.ones((8, 128, 4, HD + 1), BF)
        lv_aug[..., :HD] = long_v[b][:, hs, :].reshape(8, 128, 4, HD).astype(BF)
        wp = np.ascontiguousarray(w_proj[4 * g * 64:(4 * g + 4) * 64, :]).astype(BF)
        ind2 = np.zeros((33, 128), BF)
        ind2[0, 0:64] = 1.0
        ind2[32, 64:128] = 1.0
        in_maps.append({
            "xT": xT, "wqk": wqk, "wv": wv, "wp": wp, "lkT": lkT,
            "lv": lv_aug, "ctab": ctab.astype(BF), "stab": stab.astype(BF),
            "pswap": pswap.astype(BF),
            "tri": tri.astype(BF), "vones": np.ones((128, 48), BF),
            "ind2": ind2,
            "ind2f": ind2.astype(np.float32),
            "zeros": np.zeros((128, 768), BF),
        })
    return in_maps


def kernel(x, short_term_memory, long_q, long_k, long_v, w_attn, w_proj):
    x = np.asarray(x, np.float32)
    short_term_memory = np.asarray(short_term_memory, np.float32)
    long_k = np.asarray(long_k, np.float32)
    long_v = np.asarray(long_v, np.float32)
    w_attn = np.asarray(w_attn, np.float32)
    w_proj = np.asarray(w_proj, np.float32)

    nc = build_program()
    in_maps = prep_in_maps(x, short_term_memory, long_k, long_v, w_attn, w_proj)

    from concourse import bass_utils
    res = bass_utils.run_bass_kernel_spmd(nc, in_maps, core_ids=list(range(N_CORES)))

    out = np.zeros((B, T, C), np.float32)
    for core in range(N_CORES):
        out[core // 4] += res.results[core]["out"].astype(np.float32)
    return out
